# revision 17
# baseline (speedup 1.0000x reference)
"""Trainium2 Bass kernel for CenterHeadGroupSbnet.

Reference computation (NCHW, B=2, C=512, H=W=180):
  mp     = per-pixel mask from tile_mask [B,18,18] (10x10 px tiles)
  shared = relu(bn(conv3x3(x*mp, w_shared[64,512])))*mp
  h      = relu(bn(conv3x3(shared, w_hm1[384,64])))*mp
  hm     = (conv3x3_grouped(h, w_hm2[12,64], g=6) + bias_hm)*mp
  a      = relu(bn(conv3x3(shared, w_attr1[1920,64])))*mp
  attr   = (conv3x3_grouped(a, w_attr2[90,64], g=30) + bias_attr)*mp
  out    = concat([hm, attr], ch) -> [B, 102, 180, 180]

Sharding: 8 cores = batch (2) x row-blocks (4 x 45 rows), SPMD (identical
program, per-core input slabs, zero-padded halos so every conv is VALID).
Per core: 5 chunks of 9 output rows. Convs run as matmuls (bf16 or fp32r)
with 3x3 taps accumulated in PSUM; BN folds into the ACT eviction (relu +
per-partition scale/bias); the SBNet mask is applied with DVE multiplies.
The 64->128ch branch convs pack two taps per matmul (K=128) using a
column-shifted duplicate of `shared` on partitions 64..127. The grouped
head convs run two groups per matmul via block-diagonal K=128 weights.
"""
import sys

if '/opt/trn_rl_repo' not in sys.path:
    sys.path.insert(0, '/opt/trn_rl_repo')

from contextlib import ExitStack

import numpy as np

import concourse.bass as bass
import concourse.tile as tile
import concourse.mybir as mybir
from concourse import bacc

EPS = 1e-5
B, C, H, W = 2, 512, 180, 180
PAD = 3
HP, WP = H + 2 * PAD, W + 2 * PAD          # 186
RPC = 45                                   # output rows per core
NCHUNK = 5
RO = 9                                     # output rows per chunk
WS, WH, WO = 184, 182, 180                 # region widths per level
RS, RH, RX = 13, 11, 15                    # region rows per chunk per level
XROWS = RPC + 6                            # 51 input rows per core slab
F32R = mybir.dt.float32r
F32 = mybir.dt.float32
BF16 = mybir.dt.bfloat16

NHM = 3     # hm1 M-tiles (384/128)
NAM = 15    # attr1 M-tiles (1920/128)

import os
USE_BF16 = os.environ.get("KM_BF16", "1") == "1"   # matmul-feed dtype


def build_nc(repeat=1, use_bf16=USE_BF16):
    DT = BF16 if use_bf16 else F32R
    nc = bacc.Bacc("TRN2", target_bir_lowering=False, debug=False,
                   enable_asserts=False, num_devices=8)
    d = {}
    d['xs'] = nc.dram_tensor("xs", (C, XROWS, WP), DT, kind="ExternalInput").ap()
    d['ms'] = nc.dram_tensor("ms", (XROWS, WP), DT, kind="ExternalInput").ap()
    d['mso'] = nc.dram_tensor("mso", (XROWS, WP), F32, kind="ExternalInput").ap()
    d['w1p'] = nc.dram_tensor("w1p", (128, 36, 64), DT, kind="ExternalInput").ap()
    d['whp'] = nc.dram_tensor("whp", (128, 18, 128), DT, kind="ExternalInput").ap()
    d['wap'] = nc.dram_tensor("wap", (NAM, 128, 6, 128), DT,
                              kind="ExternalInput").ap()
    d['wg2'] = nc.dram_tensor("wg2", (128, 918), DT, kind="ExternalInput").ap()
    d['sb1'] = nc.dram_tensor("sb1", (64, 2), F32, kind="ExternalInput").ap()
    d['sha'] = nc.dram_tensor("sha", (128, 36), F32, kind="ExternalInput").ap()
    d['bi2'] = nc.dram_tensor("bi2", (128, 18), F32, kind="ExternalInput").ap()
    out_d = nc.dram_tensor("out", (102, RPC, WO), F32, kind="ExternalOutput").ap()
    if os.environ.get("KM_DEBUG"):
        for nm, shp in [("dbg_xk", (128, RX, WP)), ("dbg_mt", (128, RX, WP)),
                        ("dbg_sh", (128, RS, WS)), ("dbg_h", (128, RH, WH)),
                        ("dbg_a", (128, RH, WH))]:
            d[nm] = nc.dram_tensor(nm, shp, DT, kind="ExternalOutput").ap()

    with tile.TileContext(nc) as tc:
        with ExitStack() as ctx:
            _build_body(ctx, tc, d, out_d, repeat)
    nc.compile()
    return nc


def _build_body(ctx, tc, d, out_d, repeat):
    nc = tc.nc
    Relu = mybir.ActivationFunctionType.Relu
    Ident = mybir.ActivationFunctionType.Identity

    # Matmul-feed dtype: bf16 if the host packed bf16, else float32r.
    # (CoreSim's run_kernel allocates float32 DRAM tensors; bitcast those.)
    DT = BF16 if d['xs'].dtype == BF16 else F32R
    d = {k: (v if (k in ('sb1', 'sha', 'bi2', 'mso') or v.dtype == DT)
             else v.bitcast(F32R)) for k, v in d.items()}

    const = ctx.enter_context(tc.tile_pool(name="const", bufs=1))
    # x k-tiles and attr1 a-tiles share one pool: their live phases alternate
    xa = ctx.enter_context(tc.tile_pool(name="xa", bufs=6))
    mpool = ctx.enter_context(tc.tile_pool(name="mpool", bufs=2))
    spool = ctx.enter_context(tc.tile_pool(name="spool", bufs=2))
    hpool = ctx.enter_context(tc.tile_pool(name="hpool", bufs=3))
    wapool = ctx.enter_context(tc.tile_pool(name="wapool", bufs=3))
    opool = ctx.enter_context(tc.tile_pool(name="opool", bufs=3))
    psum = ctx.enter_context(tc.tile_pool(name="psum", bufs=1, space="PSUM"))

    # resident constants
    w1sb = const.tile([128, 36, 64], DT)
    nc.sync.dma_start(out=w1sb, in_=d['w1p'])
    whsb = const.tile([128, 18, 128], DT)
    nc.sync.dma_start(out=whsb, in_=d['whp'])
    wg2 = const.tile([128, 918], DT)
    nc.sync.dma_start(out=wg2, in_=d['wg2'])
    sb1 = const.tile([64, 2], F32)
    nc.sync.dma_start(out=sb1, in_=d['sb1'])
    sha = const.tile([128, 36], F32)
    nc.sync.dma_start(out=sha, in_=d['sha'])
    bi2 = const.tile([128, 18], F32)
    nc.sync.dma_start(out=bi2, in_=d['bi2'])

    def chunk(ch):
        x0 = RO * ch          # x-row origin of this chunk within the slab

        # ---- mask tiles, broadcast to all 128 partitions ----
        mt = mpool.tile([128, RX, WP], DT, name=f"mt{ch}", tag="mt")
        nc.sync.dma_start(out=mt, in_=bass.AP(
            tensor=d['ms'].tensor, offset=x0 * WP,
            ap=[[0, 128], [WP, RX], [1, WP]]))
        mto = mpool.tile([128, RO, WO], F32, name=f"mto{ch}", tag="mto")
        nc.sync.dma_start(out=mto, in_=bass.AP(
            tensor=d['mso'].tensor, offset=(x0 + 3) * WP + 3,
            ap=[[0, 128], [WP, RO], [1, WO]]))

        def mview(dr, dc, rows, width):
            return mt[:, dr:dr + rows, dc:dc + width]

        # ---- conv1: x[512] -> shared[64]; all 4 k-tiles resident ----
        xks = []
        for k in range(4):
            xk = xa.tile([128, RX, WP], DT, name=f"xk{ch}_{k}", tag="xa")
            nc.sync.dma_start(out=xk, in_=d['xs'][128 * k:128 * (k + 1),
                                                  x0:x0 + RX, :])
            nc.vector.tensor_mul(xk, xk, mt)
            xks.append(xk)

        shared = spool.tile([128, RS, WS], DT, name=f"sh{ch}", tag="sh")
        shf = shared.rearrange("p r c -> p (r c)")
        for j in range(7):
            jr = 2 * j if j < 6 else 11
            c1 = psum.tile([64, 2, WS], F32, name=f"c1_{ch}_{j}", tag="c1", bufs=2)
            for k in range(4):
                for t in range(9):
                    ty, tx = divmod(t, 3)
                    rhs = xks[k][:, jr + ty:jr + ty + 2, tx:tx + WS]
                    nc.tensor.matmul(c1, w1sb[:, 9 * k + t, :], rhs,
                                     start=(k == 0 and t == 0),
                                     stop=(k == 3 and t == 8),
                                     skip_group_check=True)
            nc.scalar.activation(shared[0:64, jr:jr + 2, :], c1, Relu,
                                 bias=sb1[:, 1:2], scale=sb1[:, 0:1])
            c1f = c1.rearrange("p r c -> p (r c)")
            if j == 0:
                nc.scalar.activation(shf[64:128, 0:2 * WS - 1], c1f[:, 1:2 * WS],
                                     Relu, bias=sb1[:, 1:2], scale=sb1[:, 0:1])
            else:
                nc.scalar.activation(shf[64:128, jr * WS - 1:(jr + 2) * WS - 1],
                                     c1f, Relu, bias=sb1[:, 1:2],
                                     scale=sb1[:, 0:1])
        tail = shf[64:128, RS * WS - 1:RS * WS]
        nc.vector.memset(tail.bitcast(F32) if DT == F32R else tail, 0.0)
        nc.vector.tensor_mul(shared[0:64], shared[0:64],
                             mt[0:64, 1:1 + RS, 1:1 + WS])
        nc.vector.tensor_mul(shared[64:128], shared[64:128],
                             mt[64:128, 1:1 + RS, 2:2 + WS])
        if ch == 0 and 'dbg_sh' in d:
            nc.sync.dma_start(out=d['dbg_xk'], in_=xks[0])
            nc.sync.dma_start(out=d['dbg_mt'], in_=mt)
            nc.sync.dma_start(out=d['dbg_sh'], in_=shared)

        # ---- branch conv (hm1/attr1): shared -> one 128-ch tile ----
        def branch_tile(m, wsrc, wcol, pool, tag):
            bt = pool.tile([128, RH, WH], DT, name=f"{tag}{ch}_{m}", tag=tag)
            for j in range(6):
                jr = 2 * j if j < 5 else 9
                ph = psum.tile([128, 2, WH], F32, name=f"ph{ch}_{m}_{j}",
                               tag="ph", bufs=3)
                for s in range(6):
                    dy = s if s < 3 else s - 3
                    dxb = 0 if s < 3 else 2
                    rhs = shared[:, jr + dy:jr + dy + 2, dxb:dxb + WH]
                    nc.tensor.matmul(ph, wsrc[:, wcol + s, :], rhs,
                                     start=(s == 0), stop=(s == 5),
                                     skip_group_check=True)
                nc.scalar.activation(bt[:, jr:jr + 2, :], ph, Relu,
                                     bias=sha[:, 2 * m + 1:2 * m + 2],
                                     scale=sha[:, 2 * m:2 * m + 1])
            nc.vector.tensor_mul(bt, bt, mview(2, 2, RH, WH))
            return bt

        # ---- grouped conv for one pair-tile (2 groups, block-diag K=128);
        # writes biased outputs into ostage at partitions 32*ci.. ----
        def grouped_pair(src, wbase, mo2, bcol, ost, ci, rnd):
            for j2 in range(5):
                j2r = 2 * j2 if j2 < 4 else 7
                pg = psum.tile([mo2, 2, WO], F32,
                               name=f"pg{ch}_{rnd}_{ci}_{j2}", tag="pg", bufs=3)
                for t in range(9):
                    ty, tx = divmod(t, 3)
                    rhs = src[:, j2r + ty:j2r + ty + 2, tx:tx + WO]
                    w = wg2[:, wbase + t * mo2:wbase + (t + 1) * mo2]
                    nc.tensor.matmul(pg, w, rhs, start=(t == 0), stop=(t == 8),
                                     skip_group_check=True)
                nc.scalar.activation(ost[32 * ci:32 * ci + mo2, j2r:j2r + 2, :],
                                     pg, Ident, bias=bi2[0:mo2, bcol:bcol + 1])

        def flush_ost(ost, nt, mo2, ch_of):
            nc.vector.tensor_mul(ost, ost, mto)
            for ci in range(nt):
                dst = bass.AP(tensor=out_d.tensor,
                              offset=(ch_of + mo2 * ci) * RPC * WO
                              + RO * ch * WO,
                              ap=[[RPC * WO, mo2], [WO, RO], [1, WO]])
                nc.sync.dma_start(out=dst, in_=ost[32 * ci:32 * ci + mo2])

        # hm branch: 3 pair-tiles -> channels 0..11
        ost_hm = opool.tile([128, RO, WO], F32, name=f"oshm{ch}", tag="ost")
        nc.vector.memset(ost_hm, 0.0)
        for c in range(NHM):
            hsrc = branch_tile(c, whsb, 6 * c, hpool, "h")
            if ch == 0 and c == 0 and 'dbg_h' in d:
                nc.sync.dma_start(out=d['dbg_h'], in_=hsrc)
            grouped_pair(hsrc, 810 + c * 36, 4, 15 + c, ost_hm, c, "hm")
        flush_ost(ost_hm, NHM, 4, 0)

        # attr branch: 15 pair-tiles -> channels 12..101, staged 4 per round
        for r in range(4):
            pairs = list(range(4 * r, min(4 * r + 4, NAM)))
            ost = opool.tile([128, RO, WO], F32, name=f"osa{ch}_{r}", tag="ost")
            nc.vector.memset(ost, 0.0)
            for ci, p in enumerate(pairs):
                wa = wapool.tile([128, 6, 128], DT, name=f"wa{ch}_{p}", tag="wa")
                nc.sync.dma_start(out=wa, in_=d['wap'][p])
                asrc = branch_tile(3 + p, wa, 0, xa, "xa")
                if ch == 0 and p == 0 and 'dbg_a' in d:
                    nc.sync.dma_start(out=d['dbg_a'], in_=asrc)
                grouped_pair(asrc, p * 54, 6, p, ost, ci, f"a{r}")
            flush_ost(ost, len(pairs), 6, 12 + 24 * r)

    if repeat > 1:
        with tc.For_i(0, repeat, 1):
            for ch in range(NCHUNK):
                chunk(ch)
    else:
        for ch in range(NCHUNK):
            chunk(ch)


# ---------------------------------------------------------------------------
# host side
# ---------------------------------------------------------------------------

def _bnfold(g, b, m, v):
    sc = (np.asarray(g, np.float32)
          / np.sqrt(np.asarray(v, np.float32) + EPS)).astype(np.float32)
    bi = (np.asarray(b, np.float32) - np.asarray(m, np.float32) * sc)
    return sc, bi.astype(np.float32)


def host_pack(inputs, use_bf16=USE_BF16):
    """Build the 8 per-core input maps from the full-problem inputs."""
    import ml_dtypes
    mmdt = ml_dtypes.bfloat16 if use_bf16 else np.float32

    x = np.asarray(inputs['x'], np.float32)
    tm = np.asarray(inputs['tile_mask'])

    # pixel mask, padded
    mp = np.repeat(np.repeat(tm.astype(np.float32), 10, axis=1), 10, axis=2)
    mp_pad = np.zeros((B, HP, WP), np.float32)
    mp_pad[:, PAD:PAD + H, PAD:PAD + W] = mp
    x_pad = np.zeros((B, C, HP, WP), np.float32)
    x_pad[:, :, PAD:PAD + H, PAD:PAD + W] = x

    sc_s, bi_s = _bnfold(inputs['g_s'], inputs['b_s'], inputs['m_s'], inputs['v_s'])
    sc_h, bi_h = _bnfold(inputs['g_h'], inputs['b_h'], inputs['m_h'], inputs['v_h'])
    sc_a, bi_a = _bnfold(inputs['g_a'], inputs['b_a'], inputs['m_a'], inputs['v_a'])

    w_shared = np.asarray(inputs['w_shared'], np.float32)
    w_hm1 = np.asarray(inputs['w_hm1'], np.float32)
    w_attr1 = np.asarray(inputs['w_attr1'], np.float32)
    w_hm2 = np.asarray(inputs['w_hm2'], np.float32)
    w_attr2 = np.asarray(inputs['w_attr2'], np.float32)
    bias_hm = np.asarray(inputs['bias_hm'], np.float32)
    bias_attr = np.asarray(inputs['bias_attr'], np.float32)

    # conv1 weights: w1p[ci, 9k+t, co] = w_shared[co, 128k+ci, ty, tx]
    w1p = np.ascontiguousarray(
        w_shared.reshape(64, 4, 128, 9).transpose(2, 1, 3, 0).reshape(128, 36, 64))

    def pack_pair(wfull, nm):
        # [nm*128, 64, 3, 3] -> [128, nm, 6, 128] pair-slot layout
        out = np.zeros((128, nm, 6, 128), np.float32)
        wr = wfull.reshape(nm, 128, 64, 3, 3)
        for s in range(3):
            out[0:64, :, s, :] = wr[:, :, :, s, 0].transpose(2, 0, 1)
            out[64:128, :, s, :] = wr[:, :, :, s, 1].transpose(2, 0, 1)
            out[0:64, :, 3 + s, :] = wr[:, :, :, s, 2].transpose(2, 0, 1)
        return out

    whp = pack_pair(w_hm1, NHM).reshape(128, 18, 128)
    wap = np.ascontiguousarray(
        pack_pair(w_attr1, NAM).transpose(1, 0, 2, 3))   # [15,128,6,128]

    wg2 = np.zeros((128, 918), np.float32)
    for p in range(NAM):
        for t in range(9):
            ty, tx = divmod(t, 3)
            col = (p * 9 + t) * 6
            wg2[0:64, col:col + 3] = w_attr2[6 * p:6 * p + 3, :, ty, tx].T
            wg2[64:128, col + 3:col + 6] = w_attr2[6 * p + 3:6 * p + 6, :, ty, tx].T
    for c in range(3):
        for t in range(9):
            ty, tx = divmod(t, 3)
            col = 810 + (c * 9 + t) * 4
            wg2[0:64, col:col + 2] = w_hm2[4 * c:4 * c + 2, :, ty, tx].T
            wg2[64:128, col + 2:col + 4] = w_hm2[4 * c + 2:4 * c + 4, :, ty, tx].T

    sb1 = np.stack([sc_s, bi_s], axis=1)                      # [64,2]
    sha = np.zeros((128, 36), np.float32)
    for m in range(NHM):
        sha[:, 2 * m] = sc_h[128 * m:128 * (m + 1)]
        sha[:, 2 * m + 1] = bi_h[128 * m:128 * (m + 1)]
    for p in range(NAM):
        sha[:, 2 * (3 + p)] = sc_a[128 * p:128 * (p + 1)]
        sha[:, 2 * (3 + p) + 1] = bi_a[128 * p:128 * (p + 1)]

    bi2 = np.zeros((128, 18), np.float32)
    for p in range(NAM):
        bi2[0:6, p] = bias_attr[6 * p:6 * p + 6]
    for c in range(3):
        bi2[0:4, 15 + c] = bias_hm[4 * c:4 * c + 4]

    shared_w = dict(
        w1p=w1p.astype(mmdt), whp=whp.astype(mmdt), wap=wap.astype(mmdt),
        wg2=wg2.astype(mmdt), sb1=sb1, sha=sha, bi2=bi2)
    in_maps = []
    for cidx in range(8):
        b, rb = divmod(cidx, 4)
        m = dict(shared_w)
        m['xs'] = np.ascontiguousarray(
            x_pad[b, :, RPC * rb:RPC * rb + XROWS, :]).astype(mmdt)
        msl = np.ascontiguousarray(mp_pad[b, RPC * rb:RPC * rb + XROWS, :])
        m['ms'] = msl.astype(mmdt)
        m['mso'] = msl
        in_maps.append(m)
    return in_maps


def make_runner(nc, n_cores=8):
    """Persistent jitted SPMD runner (axon PJRT path)."""
    import jax
    from jax.sharding import Mesh, PartitionSpec
    from jax.experimental.shard_map import shard_map
    from concourse import bass2jax

    bass2jax.install_neuronx_cc_hook()
    in_names, out_names, out_avals = [], [], []
    pname = nc.partition_id_tensor.name if nc.partition_id_tensor else None
    for alloc in nc.m.functions[0].allocations:
        if not isinstance(alloc, mybir.MemoryLocationSet):
            continue
        name = alloc.memorylocations[0].name
        if alloc.kind == "ExternalInput":
            if name != pname:
                in_names.append(name)
        elif alloc.kind == "ExternalOutput":
            out_names.append(name)
            out_avals.append(jax.core.ShapedArray(
                tuple(alloc.tensor_shape), mybir.dt.np(alloc.dtype)))
    n_params, n_outs = len(in_names), len(out_avals)
    all_names = in_names + out_names + ([pname] if pname else [])

    def _bass_call(*args):
        operands = list(args)
        if pname:
            operands.append(bass2jax.partition_id_tensor())
        return tuple(bass2jax._bass_exec_p.bind(
            *operands, out_avals=tuple(out_avals), in_names=tuple(all_names),
            out_names=tuple(out_names), lowering_input_output_aliases=(),
            sim_require_finite=True, sim_require_nnan=True, nc=nc))

    devices = jax.devices()[:n_cores]
    mesh = Mesh(np.array(devices), ("core",))
    fn = jax.jit(
        shard_map(_bass_call, mesh=mesh,
                  in_specs=(PartitionSpec("core"),) * (n_params + n_outs),
                  out_specs=(PartitionSpec("core"),) * n_outs,
                  check_rep=False),
        donate_argnums=tuple(range(n_params, n_params + n_outs)),
        keep_unused=True)
    return fn, in_names, out_names, out_avals


_CACHE = {}


def run_on_cores(in_maps, repeat=1, use_bf16=USE_BF16):
    """Compile (cached) and execute on the 8 cores; returns per-core outputs."""
    key = (repeat, use_bf16)
    if key not in _CACHE:
        nc = build_nc(repeat, use_bf16)
        _CACHE[key] = make_runner(nc)
    fn, in_names, out_names, out_avals = _CACHE[key]
    concat_in = [np.concatenate([np.ascontiguousarray(m[n]) for m in in_maps],
                                axis=0) for n in in_names]
    zeros = [np.zeros((8 * a.shape[0], *a.shape[1:]), a.dtype)
             for a in out_avals]
    outs = fn(*concat_in, *zeros)
    outs = [np.asarray(o) for o in outs]
    per_core = [{name: outs[i].reshape(8, *out_avals[i].shape)[c]
                 for i, name in enumerate(out_names)} for c in range(8)]
    return per_core


def kernel(**inputs) -> np.ndarray:
    in_maps = host_pack(inputs)
    per_core = run_on_cores(in_maps)
    full = np.zeros((B, 102, H, W), np.float32)
    for c in range(8):
        b, rb = divmod(c, 4)
        full[b, :, RPC * rb:RPC * rb + RPC, :] = per_core[c]['out']
    return full


if __name__ == "__main__":
    nc = build_nc()
    print("built ok")
